# revision 2
# baseline (speedup 1.0000x reference)
"""Trainium2 Bass kernel: batched single-head self-attention.

Reference computation (per (b, l) pair, 20 independent blocks):
    X = x[b, l] viewed as [N=1024, D=256] (xf layout)
    out[b, l] = softmax(beta * X @ X.T, axis=-1) @ X

Device algorithm (per block):
  * Scores: S[m, n] = sum_d X^T[d, m] X^T[d, n] on the TensorEngine with
    D on partitions -- the natural HBM layout of x[b, l] is already X^T.
    S is symmetric, so the PSUM tile doubles as the [keys, queries]
    orientation the second matmul wants: no transpose of the score
    matrix, ever.
  * Softmax shift: W[m, n] = exp(beta * (S[m, n] - c_n)) with
    c_n = ||x_n||^2 (the score diagonal -- a valid shift here since the
    attention is diagonal-dominant by ~100 nats). The per-QUERY shift
    rides the score matmul as one extra K=1 accumulation term
    (lhsT = ones row, rhs = -c row), so W comes out of a single ScalarE
    activation pass over PSUM.
  * Second matmul: computed as O^T[d, n] = sum_m xfo[m, d] W[m, n] with
    the value operand xfo = [X | 1 | 0] STATIONARY -- 3 weight loads per
    key tile instead of one per output tile, and every matmul streams
    512 columns (weight loads hide under the stream). The [1|0] chunk
    makes the softmax denominator Z_n fall out as an extra output row.
    Normalization (divide by Z) and the final [d, n] -> [n, d] layout
    flip happen on the host, where they are free.
  * Everything runs in fp32r (relaxed fp32: ~13-bit effective mantissa,
    full-rate 1 col/cycle PE streaming vs 4 cyc/col for exact fp32).
    The data has near-duplicate key pairs (diagonal-vs-offdiag score
    gaps down to -60 nats), so contested softmax rows need ~1e-2-accurate
    scores: bf16 scores are NOT enough, fp32r is. The per-query shift
    c_n rides as a bias and cancels exactly in O/Z.

Host pre/post (layout + O(N*D) work only; all O(N^2*D) flops on device):
  * xb  = X^T                    (score operands)
  * xf  = [X | 1 | 0]            (value operand)
  * negc = -||x_n||^2            (softmax shift row)
  * out = (O^T).T / Z            (normalize + layout)

Sharding: 20 blocks over 8 cores as 2 full blocks + 1 half block (512
queries) per core -- exact, no padded compute. The half blocks use a
host-side rotation of the key axis so every core runs the identical
program (softmax is invariant to key permutation when values are
permuted identically).
"""

import numpy as np
import ml_dtypes

import concourse.tile as tile
from concourse import bacc, mybir
from concourse.bass_utils import run_bass_kernel_spmd

F32 = mybir.dt.float32
F32R = mybir.dt.float32r
BF16 = mybir.dt.bfloat16

B, L, D, H, W = 4, 5, 256, 32, 32
N = H * W            # 1024 keys per block
NBLK = B * L         # 20
NCORES = 8
NFULL = 2            # full blocks per core
NSLAB = 3            # 2 full + 1 half
DF = D + 8           # value operand row: [x | 1 | 0 | pad...] -- padded to
                     # 264 floats = 1056 B so SBUF rows stay 32B-aligned
                     # (unaligned weight rows double LDWEIGHTS time)

EXP = mybir.ActivationFunctionType.Exp


def build_program(beta: float, fast: bool = True):
    mdt = F32R if fast else F32   # all matmul operands
    nc = bacc.Bacc("TRN2", target_bir_lowering=False, debug=False,
                   num_devices=NCORES)
    # Inputs are host-packed in device layout so every DMA is a plain
    # contiguous [128, *] transfer with large descriptors.
    xb_in = nc.dram_tensor("xb_in", [NSLAB, 128, 2, N], mdt,
                           kind="ExternalInput")
    xf_in = nc.dram_tensor("xf_in", [NSLAB, 128, 8, DF], mdt,
                           kind="ExternalInput")
    nc_in = nc.dram_tensor("nc_in", [1, NSLAB * N], mdt, kind="ExternalInput")
    yt_out = nc.dram_tensor("yt_out", [NSLAB, 2, 128, N], F32,
                            kind="ExternalOutput")
    z_out = nc.dram_tensor("z_out", [NSLAB, N], F32, kind="ExternalOutput")

    with tile.TileContext(nc) as tc:
        _build(tc, nc, xb_in.ap(), xf_in.ap(), nc_in.ap(), yt_out.ap(),
               z_out.ap(), beta, mdt)
    nc.finalize()
    return nc


def _build(tc, nc, xb_in, xf_in, nc_in, yt_out, z_out, beta, mdt):
    import contextlib
    ctx = contextlib.ExitStack()
    with ctx:
        const = ctx.enter_context(tc.tile_pool(name="const", bufs=1))
        xb_pool = ctx.enter_context(tc.tile_pool(name="xb", bufs=NSLAB))
        xfo_pool = ctx.enter_context(tc.tile_pool(name="xfo", bufs=NSLAB))
        negc_pool = ctx.enter_context(tc.tile_pool(name="negc", bufs=NSLAB))
        # W tiles stay live until the Z pass at the end of the block.
        w_pool = ctx.enter_context(tc.tile_pool(name="w", bufs=10))
        ot_sb_pool = ctx.enter_context(tc.tile_pool(name="ot_sb", bufs=2))
        z_sb_pool = ctx.enter_context(tc.tile_pool(name="z_sb", bufs=2))
        # PSUM: 2 score slots x 2 banks + 4 O^T accumulator banks = 8.
        # The Z-row accumulators reuse the score slots (same tag) after
        # the key loop, when the score pipeline has drained.
        ps_s = ctx.enter_context(tc.tile_pool(name="ps_s", bufs=2, space="PSUM"))
        ps_od = ctx.enter_context(tc.tile_pool(name="ps_od", bufs=4, space="PSUM"))

        ones_row_f32 = const.tile([1, 128], F32)
        nc.gpsimd.memset(ones_row_f32[:], 1.0)
        if mdt is F32:
            ones_row = ones_row_f32
        else:
            ones_row = const.tile([1, 128], mdt)
            nc.vector.tensor_copy(ones_row[:], ones_row_f32[:])

        # Warm the PE clock (HAM) with throwaway full-array fp32 matmuls
        # that run during the input-DMA window -- otherwise the first
        # ~3.4us of real matmuls run at half clock. Full 128x128 tiles:
        # small-quadrant matmuls do not register as PE activity.
        warm_src = const.tile([128, 512], F32)
        nc.gpsimd.memset(warm_src[:], 0.0)
        warm_ps = ps_od.tile([128, 512], F32, tag="od", name="warm_ps")
        for wi in range(1):
            nc.tensor.matmul(warm_ps[:], warm_src[:, 0:128], warm_src[:],
                             start=True, stop=True)

        # All input DMAs upfront. Score operands on the Sync DMA queue
        # (they gate the first matmuls), value operands + shift rows on
        # the Scalar DMA queue so the issue overheads run in parallel.
        xbs, xfos = [], []
        negc_all = negc_pool.tile([1, NSLAB * N], mdt, tag="negc")
        nc.scalar.dma_start(out=negc_all[:], in_=nc_in[:])
        negcs = [negc_all[:, s * N:(s + 1) * N] for s in range(NSLAB)]
        for s in range(NSLAB):
            xb = xb_pool.tile([128, 2, N], mdt, tag="xb", name=f"xb_{s}")
            nc.sync.dma_start(out=xb[:], in_=xb_in[s])
            xbs.append(xb)
        for s in range(NSLAB):
            xfo = xfo_pool.tile([128, 8, DF], mdt, tag="xfo",
                                name=f"xfo_{s}")
            nc.scalar.dma_start(out=xfo[:], in_=xf_in[s])
            xfos.append(xfo)

        for s in range(NSLAB):
            n_q = N if s < NFULL else N // 2
            n_h = n_q // 512    # PSUM bank halves (queries)
            xb, xfo, negc = xbs[s], xfos[s], negcs[s]

            # O^T accumulators, live across the whole key loop
            od = [[ps_od.tile([128, 512], F32, tag="od",
                              name=f"od_{s}_{ci}_{h}")
                   for h in range(n_h)] for ci in range(2)]

            w_tiles = []
            for a in range(8):      # key tile (partitions of S' and W)
                asl = slice(a * 128, (a + 1) * 128)
                # S'[m, n] = S - c_n: two data chunks + the shift term.
                # Weight-reuse order: both query halves per stationary
                # operand, so weight loads amortize over 2x512 columns.
                sps = ps_s.tile([128, N], F32, tag="sps")
                for c in range(2):
                    for h in range(n_h):
                        hs = slice(h * 512, (h + 1) * 512)
                        nc.tensor.matmul(sps[:, hs], xb[:, c, asl],
                                         xb[:, c, hs],
                                         start=(c == 0), stop=False)
                for h in range(n_h):
                    hs = slice(h * 512, (h + 1) * 512)
                    nc.tensor.matmul(sps[:, hs], ones_row[:], negc[:, hs],
                                     start=False, stop=True)
                # W[m, n] = exp(beta * S'), one ACT pass per query half
                # so the first O^T matmuls only wait on their own half
                wt = w_pool.tile([128, N], mdt, tag="w", name=f"w_{s}_{a}")
                for h in range(n_h):
                    hs = slice(h * 512, (h + 1) * 512)
                    nc.scalar.activation(wt[:, hs], sps[:, hs], EXP,
                                         scale=float(beta))
                w_tiles.append(wt)
                # O^T += xfo[a].T @ W[a]  (value operand stationary)
                for ci, csl in ((0, slice(0, 128)), (1, slice(128, 256))):
                    for h in range(n_h):
                        hs = slice(h * 512, (h + 1) * 512)
                        nc.tensor.matmul(od[ci][h][:], xfo[:, a, csl],
                                         wt[:, hs],
                                         start=(a == 0), stop=(a == 7))

            # Evacuate O^T accumulators (frees the banks for the Z pass
            # and for the next block), then run the Z pass in freed banks:
            # Z[n] = sum_m W[m, n] via the [1|0] chunk of xfo.
            ot_sb = ot_sb_pool.tile([128, 2, N], F32, tag="ot_sb")
            z_sb = z_sb_pool.tile([1, N], F32, tag="z_sb")
            for h in range(n_h):
                hs = slice(h * 512, (h + 1) * 512)
                # split the two evacuation copies across DVE and ACT so
                # they run in parallel at the block tail
                nc.vector.tensor_copy(ot_sb[:, 0, hs], od[0][h][:])
                nc.scalar.copy(ot_sb[:, 1, hs], od[1][h][:])
            oz = [ps_od.tile([128, 512], F32, tag="od", name=f"oz_{s}_{h}")
                  for h in range(n_h)]
            for a in range(8):
                for h in range(n_h):
                    hs = slice(h * 512, (h + 1) * 512)
                    nc.tensor.matmul(oz[h][0:2, 0:512], xfo[:, a, 256:258],
                                     w_tiles[a][:, hs],
                                     start=(a == 0), stop=(a == 7))
            for h in range(n_h):
                hs = slice(h * 512, (h + 1) * 512)
                nc.vector.tensor_copy(z_sb[:, hs], oz[h][0:1, 0:512])
                nc.sync.dma_start(
                    out=yt_out[s][:, :, hs].rearrange("c p n -> p c n"),
                    in_=ot_sb[:, :, hs])
            nc.sync.dma_start(out=z_out[s][:n_q].unsqueeze(0),
                              in_=z_sb[:, :n_q])


_PROG_CACHE = {}


def _get_program(beta: float, fast: bool = True):
    key = (beta, fast)
    if key not in _PROG_CACHE:
        _PROG_CACHE[key] = build_program(beta, fast)
    return _PROG_CACHE[key]


def make_in_maps(x: np.ndarray, fast: bool = True):
    """Shard the full input [B, L, D, H, W] into 8 per-core input maps."""
    xt_all = np.ascontiguousarray(x.reshape(NBLK, D, N))
    in_maps = []
    for c in range(NCORES):
        half_blk = NFULL * NCORES + c // 2
        half = xt_all[half_blk]
        if c % 2 == 1:
            # rotate keys so this core's queries are columns 0..511
            half = np.concatenate([half[:, N // 2:], half[:, :N // 2]], axis=1)
        slabs = np.stack([xt_all[NFULL * c], xt_all[NFULL * c + 1], half])
        xf = np.zeros((NSLAB, N, DF), np.float32)
        xf[:, :, :D] = slabs.transpose(0, 2, 1)
        xf[:, :, D] = 1.0
        negc = -np.einsum('sdn,sdn->sn', slabs, slabs)
        # pack into device layout: xb [128, 2, N], xf [128, 8, DF]
        xb_p = slabs.reshape(NSLAB, 2, 128, N).transpose(0, 2, 1, 3)
        xf_p = xf.reshape(NSLAB, 8, 128, DF).transpose(0, 2, 1, 3)
        in_maps.append({"xb_in": np.ascontiguousarray(xb_p),
                        "xf_in": np.ascontiguousarray(xf_p),
                        "nc_in": np.ascontiguousarray(
                            negc.reshape(1, NSLAB * N))})
    return in_maps


def assemble_output(results):
    """Normalize, transpose and gather per-core outputs into [B, L, N, D]."""
    out = np.empty((NBLK, N, D), np.float32)
    for c in range(NCORES):
        yt = results[c]["yt_out"].reshape(NSLAB, 2 * 128, N)
        z = results[c]["z_out"]
        for s, blk, lo, n_q in ((0, NFULL * c, 0, N),
                                (1, NFULL * c + 1, 0, N),
                                (2, NFULL * NCORES + c // 2,
                                 (c % 2) * (N // 2), N // 2)):
            ot = yt[s, :, :n_q]                       # [D, n_q], unnormalized
            out[blk, lo:lo + n_q] = (ot / z[s, :n_q]).T
    return out.reshape(B, L, N, D)


def kernel(x, beta, _trace=False, _fast=True, _tmpdir=None):
    x = np.asarray(x, dtype=np.float32)
    assert x.shape == (B, L, D, H, W), x.shape
    beta_f = float(np.asarray(beta))
    prog = _get_program(beta_f, _fast)
    in_maps = make_in_maps(x, _fast)
    res = run_bass_kernel_spmd(prog, in_maps, core_ids=list(range(NCORES)),
                               trace=_trace, tmpdir=_tmpdir)
    out = assemble_output(res.results)
    if _trace:
        return out, res
    return out



# revision 5
# speedup vs baseline: 1.3859x; 1.3859x over previous
"""Trainium2 Bass kernel: batched single-head self-attention.

Reference computation (per (b, l) pair, 20 independent blocks):
    X = x[b, l] viewed as [N=1024, D=256] (xf layout)
    out[b, l] = softmax(beta * X @ X.T, axis=-1) @ X

Device algorithm (per block):
  * Scores: S[m, n] = sum_d X^T[d, m] X^T[d, n] on the TensorEngine with
    D on partitions. S is symmetric, so the PSUM tile doubles as the
    [keys, queries] orientation the second matmul wants.
  * Softmax shift: W[m, n] = exp(beta * (S[m, n] - c_n)) with
    c_n = ||x_n||^2. The per-QUERY shift rides the score matmul as one
    extra K=1 accumulation term (lhsT = ones row, rhs = -c row).
  * Second matmul: O^T[d, n] = sum_m xfo[m, d] W[m, n] with the value
    operand xfo = [X | 1 | 0] stationary. The [1|0] chunk gives the
    softmax denominator Z_n as a separate accumulation pass.
    Normalization (divide by Z) and the [d, n] -> [n, d] flip happen on
    the host.
  * Dtypes: score operands fp16 (10-bit mantissa; measured end-to-end
    rel err ~3e-3 vs the 2e-2 gate), W tiles and values bf16 (W must be
    bf16 not fp16: exp(S - c_n) reaches e^60, past fp16 range). 16-bit
    weights let every LDWEIGHTS use the fast weight-load path -- fp32
    weight loads (~224 ns per 128x128, no FWL) were the cadence limiter
    of the fp32r version (~275 ns per 512-col matmul vs 213 ns ideal).
  * Software pipelining: the AV matmuls of key tile a are emitted after
    the score matmuls of tile a+1, so ScalarE's exp of tile a runs under
    the tile-a+1 score streams and the PE never waits on ACT.
  * The Z pass of block s is emitted between the first and second score
    tiles of block s+1 so the PSUM evacuation it depends on overlaps
    score streaming instead of stalling the PE.

Sharding: 20 blocks over 8 cores as 2 full blocks + 1 half block (512
queries) per core. The half blocks use a host-side rotation of the key
axis so every core runs the identical program (softmax is invariant to
key permutation when values are permuted identically).
"""

import numpy as np
import ml_dtypes

import concourse.tile as tile
from concourse import bacc, mybir
from concourse.bass_utils import run_bass_kernel_spmd

F32 = mybir.dt.float32
F16 = mybir.dt.float16
BF16 = mybir.dt.bfloat16

B, L, D, H, W = 4, 5, 256, 32, 32
N = H * W            # 1024 keys per block
NBLK = B * L         # 20
NCORES = 8
NFULL = 2            # full blocks per core
NSLAB = 3            # 2 full + 1 half
DF = 272             # value operand row: [x(256) | 1 | 0...] padded so bf16
                     # rows are 544 B = 17x32 B (32 B-aligned weight rows)

EXP = mybir.ActivationFunctionType.Exp


def build_program(beta: float, fast: bool = True):
    sdt = F16 if fast else F32    # score operand dtype
    wdt = BF16 if fast else F32   # W tiles / value operand dtype
    # bf16, not fp16: the UNNORMALIZED O^T rows reach ~e^60 on contested
    # softmax rows (W = exp(S - c_n) can exceed fp16 range before the
    # host-side divide by Z)
    odt = BF16 if fast else F32   # output dtype
    nc = bacc.Bacc("TRN2", target_bir_lowering=False, debug=False,
                   num_devices=NCORES)
    # Inputs are host-packed in device layout so every DMA is a plain
    # contiguous transfer with large descriptors.
    xb_in = nc.dram_tensor("xb_in", [NSLAB, 2, 128, N], sdt,
                           kind="ExternalInput")
    xf_in = nc.dram_tensor("xf_in", [NSLAB, 128, 8, DF], wdt,
                           kind="ExternalInput")
    nc_in = nc.dram_tensor("nc_in", [1, NSLAB * N], sdt, kind="ExternalInput")
    yt_out = nc.dram_tensor("yt_out", [NSLAB, 128, 2, N], odt,
                            kind="ExternalOutput")
    z_out = nc.dram_tensor("z_out", [NSLAB, N], F32, kind="ExternalOutput")

    with tile.TileContext(nc) as tc:
        _build(tc, nc, xb_in.ap(), xf_in.ap(), nc_in.ap(), yt_out.ap(),
               z_out.ap(), beta, sdt, wdt, odt)
    nc.finalize()
    return nc


def _build(tc, nc, xb_in, xf_in, nc_in, yt_out, z_out, beta, sdt, wdt, odt):
    import contextlib
    ctx = contextlib.ExitStack()
    with ctx:
        const = ctx.enter_context(tc.tile_pool(name="const", bufs=1))
        xb_pool = ctx.enter_context(tc.tile_pool(name="xb", bufs=NSLAB))
        xfo_pool = ctx.enter_context(tc.tile_pool(name="xfo", bufs=NSLAB))
        negc_pool = ctx.enter_context(tc.tile_pool(name="negc", bufs=NSLAB))
        # W tiles stay live until the Z pass at the end of the block.
        w_pool = ctx.enter_context(tc.tile_pool(name="w", bufs=10))
        ot_sb_pool = ctx.enter_context(tc.tile_pool(name="ot_sb", bufs=2))
        z_sb_pool = ctx.enter_context(tc.tile_pool(name="z_sb", bufs=2))
        # PSUM: 2 score slots x 2 banks + 4 O^T accumulator banks = 8.
        # The Z-row accumulators reuse the O^T slots (same tag) after the
        # key loop, once the block's accumulators are evacuated.
        ps_s = ctx.enter_context(tc.tile_pool(name="ps_s", bufs=2, space="PSUM"))
        ps_od = ctx.enter_context(tc.tile_pool(name="ps_od", bufs=4, space="PSUM"))

        ones_row_f32 = const.tile([1, 128], F32)
        nc.gpsimd.memset(ones_row_f32[:], 1.0)
        if sdt is F32:
            ones_row = ones_row_f32
        else:
            ones_row = const.tile([1, 128], sdt)
            nc.vector.tensor_copy(ones_row[:], ones_row_f32[:])

        # Warm the PE clock (HAM) with throwaway matmuls that run during
        # the input-DMA window -- otherwise the first ~3.4us of real
        # matmuls run at half clock.
        warm_src = const.tile([128, 512], sdt)
        nc.gpsimd.memset(warm_src[:], 0.0)
        warm_ps = ps_od.tile([128, 512], F32, tag="od", name="warm_ps")
        for wi in range(4):
            nc.tensor.matmul(warm_ps[:], warm_src[:, 0:128], warm_src[:],
                             start=True, stop=True)

        # All input DMAs upfront. Score operands on the Sync DMA queue
        # (they gate the first matmuls), split per (slab, chunk) so the
        # first score matmuls only wait for the first 0.5 MB. Value
        # operands + shift rows on the Scalar DMA queue in parallel.
        xbs, xfos = [], []
        negc_all = negc_pool.tile([1, NSLAB * N], sdt, tag="negc")
        nc.scalar.dma_start(out=negc_all[:], in_=nc_in[:])
        negcs = [negc_all[:, s * N:(s + 1) * N] for s in range(NSLAB)]
        for s in range(NSLAB):
            xb = xb_pool.tile([128, 2, N], sdt, tag="xb", name=f"xb_{s}")
            for c in range(2):
                nc.sync.dma_start(out=xb[:, c, :], in_=xb_in[s][c])
            xbs.append(xb)
        for s in range(NSLAB):
            xfo = xfo_pool.tile([128, 8, DF], wdt, tag="xfo",
                                name=f"xfo_{s}")
            nc.scalar.dma_start(out=xfo[:], in_=xf_in[s])
            xfos.append(xfo)

        def emit_scores(s, a, n_h):
            """Score matmuls + shift rider + exp for key tile a of slab s.
            Returns the W tile."""
            xb, negc = xbs[s], negcs[s]
            asl = slice(a * 128, (a + 1) * 128)
            sps = ps_s.tile([128, N], F32, tag="sps", name=f"sps_{s}_{a}")
            for c in range(2):
                for h in range(n_h):
                    hs = slice(h * 512, (h + 1) * 512)
                    nc.tensor.matmul(sps[:, hs], xb[:, c, asl], xb[:, c, hs],
                                     start=(c == 0), stop=False)
            for h in range(n_h):
                hs = slice(h * 512, (h + 1) * 512)
                nc.tensor.matmul(sps[:, hs], ones_row[:], negc[:, hs],
                                 start=False, stop=True)
            # W = exp(beta * S'), one ACT pass per query half so the first
            # AV matmuls only wait on their own half.
            wt = w_pool.tile([128, N], wdt, tag="w", name=f"w_{s}_{a}")
            for h in range(n_h):
                hs = slice(h * 512, (h + 1) * 512)
                nc.scalar.activation(wt[:, hs], sps[:, hs], EXP,
                                     scale=float(beta))
            return wt

        def emit_av(s, a, n_h, od, wt):
            """O^T += xfo[a].T @ W[a] (value operand stationary)."""
            xfo = xfos[s]
            for ci, csl in ((0, slice(0, 128)), (1, slice(128, 256))):
                for h in range(n_h):
                    hs = slice(h * 512, (h + 1) * 512)
                    nc.tensor.matmul(od[ci][h][:], xfo[:, a, csl],
                                     wt[:, hs],
                                     start=(a == 0), stop=(a == 7))

        # Per-slab state for the software pipeline.
        pend = []   # deferred work units, at most 1 deep

        def make_block(s):
            n_q = N if s < NFULL else N // 2
            n_h = n_q // 512
            od = None
            w_tiles = []

            def step(a):
                nonlocal od
                wt = emit_scores(s, a, n_h)
                w_tiles.append(wt)
                def av():
                    nonlocal od
                    if od is None:
                        od = [[ps_od.tile([128, 512], F32, tag="od",
                                          name=f"od_{s}_{ci}_{h}")
                               for h in range(n_h)] for ci in range(2)]
                    emit_av(s, a, n_h, od, wt)
                return av

            def tail():
                # Evacuate O^T accumulators (frees the banks for the Z
                # pass and for the next block). Split the copies across
                # DVE and ACT so they run in parallel.
                ot_sb = ot_sb_pool.tile([128, 2, N], odt, tag="ot_sb",
                                        name=f"ot_{s}")
                for h in range(n_h):
                    hs = slice(h * 512, (h + 1) * 512)
                    nc.vector.tensor_copy(ot_sb[:, 0, hs], od[0][h][:])
                    nc.scalar.copy(ot_sb[:, 1, hs], od[1][h][:])
                return ot_sb

            def zpass(ot_sb):
                # Z[n] = sum_m W[m, n] via the [1|0] chunk of xfo, into
                # banks freed by the evacuation.
                xfo = xfos[s]
                z_sb = z_sb_pool.tile([1, N], F32, tag="z_sb",
                                      name=f"z_{s}")
                oz = [ps_od.tile([128, 512], F32, tag="od",
                                 name=f"oz_{s}_{h}") for h in range(n_h)]
                for a in range(8):
                    for h in range(n_h):
                        hs = slice(h * 512, (h + 1) * 512)
                        nc.tensor.matmul(oz[h][0:2, 0:512],
                                         xfo[:, a, 256:258],
                                         w_tiles[a][:, hs],
                                         start=(a == 0), stop=(a == 7))
                for h in range(n_h):
                    hs = slice(h * 512, (h + 1) * 512)
                    # split across DVE and ACT so the copies run in
                    # parallel and release the banks sooner
                    if h == 0:
                        nc.vector.tensor_copy(z_sb[:, hs], oz[h][0:1, 0:512])
                    else:
                        nc.scalar.copy(z_sb[:, hs], oz[h][0:1, 0:512])
                nc.sync.dma_start(out=yt_out[s], in_=ot_sb[:])
                nc.sync.dma_start(out=z_out[s][:n_q].unsqueeze(0),
                                  in_=z_sb[:, :n_q])

            return step, tail, zpass

        # Emission order (PE program order), 1-deep pipelined:
        #   scores(s,0) scores(s,1) av(s,0) scores(s,2) av(s,1) ...
        #   scores(s,7) av(s,6) [evac via deps] av(s,7)
        #   scores(s+1,0) zpass(s) scores(s+1,1) av(s+1,0) ...
        prev_tail = None          # () emitting zpass of previous slab
        for s in range(NSLAB):
            step, tail, zpass = make_block(s)
            pend_av = None
            for a in range(8):
                av = step(a)
                if a == 1 and prev_tail is not None:
                    prev_tail()
                    prev_tail = None
                if pend_av is not None:
                    pend_av()
                pend_av = av
            pend_av()
            ot_sb = tail()
            prev_tail = (lambda zp=zpass, ot=ot_sb: zp(ot))
        prev_tail()


_PROG_CACHE = {}


def _get_program(beta: float, fast: bool = True):
    key = (beta, fast)
    if key not in _PROG_CACHE:
        _PROG_CACHE[key] = build_program(beta, fast)
    return _PROG_CACHE[key]


def make_in_maps(x: np.ndarray, fast: bool = True):
    """Shard the full input [B, L, D, H, W] into 8 per-core input maps."""
    sdt = np.float16 if fast else np.float32
    wdt = ml_dtypes.bfloat16 if fast else np.float32
    xt_all = np.ascontiguousarray(x.reshape(NBLK, D, N))
    in_maps = []
    for c in range(NCORES):
        half_blk = NFULL * NCORES + c // 2
        half = xt_all[half_blk]
        if c % 2 == 1:
            # rotate keys so this core's queries are columns 0..511
            half = np.concatenate([half[:, N // 2:], half[:, :N // 2]], axis=1)
        slabs = np.stack([xt_all[NFULL * c], xt_all[NFULL * c + 1], half])
        slabs16 = slabs.astype(sdt)
        # shift row from the rounded operands (any per-query shift cancels
        # exactly in O/Z; using the rounded data keeps the overflow margin)
        s32 = slabs16.astype(np.float32)
        negc = -np.einsum('sdn,sdn->sn', s32, s32)
        xf = np.zeros((NSLAB, N, DF), np.float32)
        xf[:, :, :D] = slabs.transpose(0, 2, 1)
        xf[:, :, D] = 1.0
        # pack into device layout: xb [2, 128, N], xf [128, 8, DF]
        xb_p = slabs16.reshape(NSLAB, 2, 128, N)
        xf_p = np.ascontiguousarray(
            xf.reshape(NSLAB, 8, 128, DF).transpose(0, 2, 1, 3)).astype(wdt)
        in_maps.append({"xb_in": np.ascontiguousarray(xb_p),
                        "xf_in": xf_p,
                        "nc_in": np.ascontiguousarray(
                            negc.reshape(1, NSLAB * N)).astype(sdt)})
    return in_maps


def assemble_output(results):
    """Normalize, transpose and gather per-core outputs into [B, L, N, D]."""
    out = np.empty((NBLK, N, D), np.float32)
    for c in range(NCORES):
        # yt [NSLAB, 128, 2, N]: partition p, value chunk ci -> O^T row
        # ci*128 + p
        yt = np.asarray(results[c]["yt_out"], dtype=np.float32)
        yt = yt.transpose(0, 2, 1, 3).reshape(NSLAB, 2 * 128, N)
        z = results[c]["z_out"]
        for s, blk, lo, n_q in ((0, NFULL * c, 0, N),
                                (1, NFULL * c + 1, 0, N),
                                (2, NFULL * NCORES + c // 2,
                                 (c % 2) * (N // 2), N // 2)):
            ot = yt[s, :, :n_q]                       # [D, n_q], unnormalized
            out[blk, lo:lo + n_q] = (ot / z[s, :n_q]).T
    return out.reshape(B, L, N, D)


def kernel(x, beta, _trace=False, _fast=True, _tmpdir=None):
    x = np.asarray(x, dtype=np.float32)
    assert x.shape == (B, L, D, H, W), x.shape
    beta_f = float(np.asarray(beta))
    prog = _get_program(beta_f, _fast)
    in_maps = make_in_maps(x, _fast)
    res = run_bass_kernel_spmd(prog, in_maps, core_ids=list(range(NCORES)),
                               trace=_trace, tmpdir=_tmpdir)
    out = assemble_output(res.results)
    if _trace:
        return out, res
    return out


# revision 9
# speedup vs baseline: 1.4259x; 1.0289x over previous
"""Trainium2 Bass kernel: batched single-head self-attention.

Reference computation (per (b, l) pair, 20 independent blocks):
    X = x[b, l] viewed as [N=1024, D=256] (xf layout)
    out[b, l] = softmax(beta * X @ X.T, axis=-1) @ X

Device algorithm (per block):
  * Scores: S[m, n] = sum_d X^T[d, m] X^T[d, n] on the TensorEngine with
    D on partitions. S is symmetric, so the PSUM tile doubles as the
    [keys, queries] orientation the second matmul wants.
  * Softmax shift: W[m, n] = exp(beta * (S[m, n] - c_n)) with
    c_n = ||x_n||^2. The per-QUERY shift rides the score matmul as one
    extra K=1 accumulation term (lhsT = ones row, rhs = -c row).
  * Second matmul: O^T[d, n] = sum_m xfo[m, d] W[m, n] with the value
    operand xfo = [X | 1 | 0] stationary. The [1|0] chunk gives the
    softmax denominator Z_n as a separate accumulation pass.
    Normalization (divide by Z) and the [d, n] -> [n, d] flip happen on
    the host.
  * Dtypes: score operands fp16 (10-bit mantissa; measured end-to-end
    rel err ~3e-3 vs the 2e-2 gate), W tiles and values bf16 (W must be
    bf16 not fp16: exp(S - c_n) reaches e^60, past fp16 range). 16-bit
    weights let every LDWEIGHTS use the fast weight-load path -- fp32
    weight loads (~224 ns per 128x128, no FWL) were the cadence limiter
    of the fp32r version (~275 ns per 512-col matmul vs 213 ns ideal).
  * Software pipelining: the AV matmuls of key tile a are emitted after
    the score matmuls of tile a+1, so ScalarE's exp of tile a runs under
    the tile-a+1 score streams and the PE never waits on ACT.
  * The Z pass of block s is emitted between the first and second score
    tiles of block s+1 so the PSUM evacuation it depends on overlaps
    score streaming instead of stalling the PE.

Sharding: 20 blocks over 8 cores as 2 full blocks + 1 half block (512
queries) per core. The half blocks use a host-side rotation of the key
axis so every core runs the identical program (softmax is invariant to
key permutation when values are permuted identically).
"""

import numpy as np
import ml_dtypes

import concourse.tile as tile
from concourse import bacc, mybir
from concourse.bass_utils import run_bass_kernel_spmd

F32 = mybir.dt.float32
F16 = mybir.dt.float16
BF16 = mybir.dt.bfloat16

B, L, D, H, W = 4, 5, 256, 32, 32
N = H * W            # 1024 keys per block
NBLK = B * L         # 20
NCORES = 8
NFULL = 2            # full blocks per core
NSLAB = 3            # 2 full + 1 half
DF = 272             # value operand row: [x(256) | 1 | 0...] padded so bf16
                     # rows are 544 B = 17x32 B (32 B-aligned weight rows)

EXP = mybir.ActivationFunctionType.Exp


def build_program(beta: float, fast: bool = True):
    sdt = F16 if fast else F32    # score operand dtype
    wdt = BF16 if fast else F32   # W tiles / value operand dtype
    # bf16, not fp16: the UNNORMALIZED O^T rows reach ~e^60 on contested
    # softmax rows (W = exp(S - c_n) can exceed fp16 range before the
    # host-side divide by Z)
    odt = BF16 if fast else F32   # output dtype
    nc = bacc.Bacc("TRN2", target_bir_lowering=False, debug=False,
                   num_devices=NCORES)
    # Inputs are host-packed in device layout so every DMA is a plain
    # contiguous transfer with large descriptors.
    xb_in = nc.dram_tensor("xb_in", [NSLAB, 2, 128, N], sdt,
                           kind="ExternalInput")
    xf_in = nc.dram_tensor("xf_in", [NSLAB, 128, 8, DF], wdt,
                           kind="ExternalInput")
    nc_in = nc.dram_tensor("nc_in", [1, NSLAB * N], sdt, kind="ExternalInput")
    yt_out = nc.dram_tensor("yt_out", [NSLAB, 128, 2, N], odt,
                            kind="ExternalOutput")
    z_out = nc.dram_tensor("z_out", [NSLAB, N], F32, kind="ExternalOutput")

    with tile.TileContext(nc) as tc:
        _build(tc, nc, xb_in.ap(), xf_in.ap(), nc_in.ap(), yt_out.ap(),
               z_out.ap(), beta, sdt, wdt, odt)
    nc.finalize()
    return nc


def _build(tc, nc, xb_in, xf_in, nc_in, yt_out, z_out, beta, sdt, wdt, odt):
    import contextlib
    ctx = contextlib.ExitStack()
    with ctx:
        const = ctx.enter_context(tc.tile_pool(name="const", bufs=1))
        xb_pool = ctx.enter_context(tc.tile_pool(name="xb", bufs=NSLAB))
        xfo_pool = ctx.enter_context(tc.tile_pool(name="xfo", bufs=NSLAB))
        negc_pool = ctx.enter_context(tc.tile_pool(name="negc", bufs=NSLAB))
        # W tiles stay live until the Z pass at the end of the block.
        w_pool = ctx.enter_context(tc.tile_pool(name="w", bufs=10))
        ot_sb_pool = ctx.enter_context(tc.tile_pool(name="ot_sb", bufs=2))
        z_sb_pool = ctx.enter_context(tc.tile_pool(name="z_sb", bufs=2))
        # PSUM: 2 score slots x 2 banks + 4 O^T accumulator banks = 8.
        # The Z-row accumulators reuse the O^T slots (same tag) after the
        # key loop, once the block's accumulators are evacuated.
        ps_s = ctx.enter_context(tc.tile_pool(name="ps_s", bufs=2, space="PSUM"))
        ps_od = ctx.enter_context(tc.tile_pool(name="ps_od", bufs=4, space="PSUM"))

        # Shift-rider stationary: a K=128 full-array operand with
        # partition 0 all-ones, rest zero, so lhsT.T @ negcb broadcasts
        # partition 0 of the moving operand to all 128 output rows.
        # A K=1 ones-row works too but a 1x128 (row-group-masked) matmul
        # forces a full PE drain before the next full-array LDWEIGHTS --
        # measured ~430 ns stall per rider.
        e1 = const.tile([128, 128], sdt)
        nc.gpsimd.memset(e1[:], 0.0)
        nc.gpsimd.memset(e1[0:1, :], 1.0)

        # Warm the PE clock (HAM) with throwaway matmuls that run during
        # the input-DMA window -- otherwise the first ~3.4us of real
        # matmuls run at half clock. Short 128-col streams keep the PE
        # queue shallow so the first real matmul isn't delayed.
        warm_ps = ps_od.tile([128, 512], F32, tag="od", name="warm_ps")
        for wi in range(8):
            nc.tensor.matmul(warm_ps[:, 0:128], e1[:], e1[:],
                             start=True, stop=True)

        # All input DMAs upfront. Score operands on the Sync DMA queue
        # (they gate the first matmuls), split per (slab, chunk) so the
        # first score matmuls only wait for the first 0.5 MB. Value
        # operands + shift rows on the Scalar DMA queue in parallel.
        xbs, xfos = [], []
        # negcb: shift rows on partition 0, partitions 1..127 zeroed (they
        # meet zero weights in the rider, but NaN garbage x 0 = NaN).
        negcb = negc_pool.tile([128, NSLAB * N], sdt, tag="negc")
        nc.vector.memset(negcb[:], 0.0)
        nc.scalar.dma_start(out=negcb[0:1, :], in_=nc_in[:])
        negcs = [negcb[:, s * N:(s + 1) * N] for s in range(NSLAB)]
        for s in range(NSLAB):
            xb = xb_pool.tile([128, 2, N], sdt, tag="xb", name=f"xb_{s}")
            for c in range(2):
                nc.sync.dma_start(out=xb[:, c, :], in_=xb_in[s][c])
            xbs.append(xb)
        for s in range(NSLAB):
            xfo = xfo_pool.tile([128, 8, DF], wdt, tag="xfo",
                                name=f"xfo_{s}")
            nc.scalar.dma_start(out=xfo[:], in_=xf_in[s])
            xfos.append(xfo)

        def emit_scores(s, a, n_h):
            """Score matmuls + shift rider + exp for key tile a of slab s.
            Returns the W tile."""
            xb, negc = xbs[s], negcs[s]
            asl = slice(a * 128, (a + 1) * 128)
            sps = ps_s.tile([128, N], F32, tag="sps", name=f"sps_{s}_{a}")
            for c in range(2):
                for h in range(n_h):
                    hs = slice(h * 512, (h + 1) * 512)
                    nc.tensor.matmul(sps[:, hs], xb[:, c, asl], xb[:, c, hs],
                                     start=(c == 0), stop=False)
            for h in range(n_h):
                hs = slice(h * 512, (h + 1) * 512)
                nc.tensor.matmul(sps[:, hs], e1[:], negc[:, hs],
                                 start=False, stop=True)
            # W = exp(beta * S'), one ACT pass per query half so the first
            # AV matmuls only wait on their own half.
            wt = w_pool.tile([128, N], wdt, tag="w", name=f"w_{s}_{a}")
            for h in range(n_h):
                hs = slice(h * 512, (h + 1) * 512)
                nc.scalar.activation(wt[:, hs], sps[:, hs], EXP,
                                     scale=float(beta))
            return wt

        def emit_av(s, a, n_h, od, wt):
            """O^T += xfo[a].T @ W[a] (value operand stationary)."""
            xfo = xfos[s]
            for ci, csl in ((0, slice(0, 128)), (1, slice(128, 256))):
                for h in range(n_h):
                    hs = slice(h * 512, (h + 1) * 512)
                    nc.tensor.matmul(od[ci][h][:], xfo[:, a, csl],
                                     wt[:, hs],
                                     start=(a == 0), stop=(a == 7))

        # Per-slab state for the software pipeline.
        pend = []   # deferred work units, at most 1 deep

        def make_block(s):
            n_q = N if s < NFULL else N // 2
            n_h = n_q // 512
            od = None
            w_tiles = []

            def step(a):
                nonlocal od
                wt = emit_scores(s, a, n_h)
                w_tiles.append(wt)
                def av():
                    nonlocal od
                    if od is None:
                        od = [[ps_od.tile([128, 512], F32, tag="od",
                                          name=f"od_{s}_{ci}_{h}")
                               for h in range(n_h)] for ci in range(2)]
                    emit_av(s, a, n_h, od, wt)
                return av

            def tail():
                # Evacuate O^T accumulators (frees the banks for the Z
                # pass and for the next block). Split the copies across
                # DVE and ACT so they run in parallel.
                ot_sb = ot_sb_pool.tile([128, 2, N], odt, tag="ot_sb",
                                        name=f"ot_{s}")
                for h in range(n_h):
                    hs = slice(h * 512, (h + 1) * 512)
                    nc.vector.tensor_copy(ot_sb[:, 0, hs], od[0][h][:])
                    nc.scalar.copy(ot_sb[:, 1, hs], od[1][h][:])
                return ot_sb

            def zpass(ot_sb):
                # Z[n] = sum_m W[m, n] via the [1|0] chunk of xfo, into
                # banks freed by the evacuation.
                xfo = xfos[s]
                z_sb = z_sb_pool.tile([1, N], F32, tag="z_sb",
                                      name=f"z_{s}")
                oz = [ps_od.tile([128, 512], F32, tag="od",
                                 name=f"oz_{s}_{h}") for h in range(n_h)]
                for a in range(8):
                    for h in range(n_h):
                        hs = slice(h * 512, (h + 1) * 512)
                        nc.tensor.matmul(oz[h][0:2, 0:512],
                                         xfo[:, a, 256:258],
                                         w_tiles[a][:, hs],
                                         start=(a == 0), stop=(a == 7))
                for h in range(n_h):
                    hs = slice(h * 512, (h + 1) * 512)
                    # split across DVE and ACT so the copies run in
                    # parallel and release the banks sooner
                    if h == 0:
                        nc.vector.tensor_copy(z_sb[:, hs], oz[h][0:1, 0:512])
                    else:
                        nc.scalar.copy(z_sb[:, hs], oz[h][0:1, 0:512])
                nc.sync.dma_start(out=yt_out[s], in_=ot_sb[:])
                nc.sync.dma_start(out=z_out[s][:n_q].unsqueeze(0),
                                  in_=z_sb[:, :n_q])

            return step, tail, zpass

        # Emission order (PE program order), 1-deep pipelined:
        #   scores(s,0) scores(s,1) av(s,0) scores(s,2) av(s,1) ...
        #   scores(s,7) av(s,6) [evac via deps] av(s,7)
        #   scores(s+1,0) zpass(s) scores(s+1,1) av(s+1,0) ...
        prev_tail = None          # () emitting zpass of previous slab
        for s in range(NSLAB):
            step, tail, zpass = make_block(s)
            pend_av = None
            for a in range(8):
                av = step(a)
                if a == 1 and prev_tail is not None:
                    prev_tail()
                    prev_tail = None
                if pend_av is not None:
                    pend_av()
                pend_av = av
            pend_av()
            ot_sb = tail()
            prev_tail = (lambda zp=zpass, ot=ot_sb: zp(ot))
        prev_tail()


_PROG_CACHE = {}


def _get_program(beta: float, fast: bool = True):
    key = (beta, fast)
    if key not in _PROG_CACHE:
        _PROG_CACHE[key] = build_program(beta, fast)
    return _PROG_CACHE[key]


def make_in_maps(x: np.ndarray, fast: bool = True):
    """Shard the full input [B, L, D, H, W] into 8 per-core input maps."""
    sdt = np.float16 if fast else np.float32
    wdt = ml_dtypes.bfloat16 if fast else np.float32
    xt_all = np.ascontiguousarray(x.reshape(NBLK, D, N))
    in_maps = []
    for c in range(NCORES):
        half_blk = NFULL * NCORES + c // 2
        half = xt_all[half_blk]
        if c % 2 == 1:
            # rotate keys so this core's queries are columns 0..511
            half = np.concatenate([half[:, N // 2:], half[:, :N // 2]], axis=1)
        slabs = np.stack([xt_all[NFULL * c], xt_all[NFULL * c + 1], half])
        slabs16 = slabs.astype(sdt)
        # shift row from the rounded operands (any per-query shift cancels
        # exactly in O/Z; using the rounded data keeps the overflow margin)
        s32 = slabs16.astype(np.float32)
        negc = -np.einsum('sdn,sdn->sn', s32, s32)
        xf = np.zeros((NSLAB, N, DF), np.float32)
        xf[:, :, :D] = slabs.transpose(0, 2, 1)
        xf[:, :, D] = 1.0
        # pack into device layout: xb [2, 128, N], xf [128, 8, DF]
        xb_p = slabs16.reshape(NSLAB, 2, 128, N)
        xf_p = np.ascontiguousarray(
            xf.reshape(NSLAB, 8, 128, DF).transpose(0, 2, 1, 3)).astype(wdt)
        in_maps.append({"xb_in": np.ascontiguousarray(xb_p),
                        "xf_in": xf_p,
                        "nc_in": np.ascontiguousarray(
                            negc.reshape(1, NSLAB * N)).astype(sdt)})
    return in_maps


def assemble_output(results):
    """Normalize, transpose and gather per-core outputs into [B, L, N, D]."""
    out = np.empty((NBLK, N, D), np.float32)
    for c in range(NCORES):
        # yt [NSLAB, 128, 2, N]: partition p, value chunk ci -> O^T row
        # ci*128 + p
        yt = np.asarray(results[c]["yt_out"], dtype=np.float32)
        yt = yt.transpose(0, 2, 1, 3).reshape(NSLAB, 2 * 128, N)
        z = results[c]["z_out"]
        for s, blk, lo, n_q in ((0, NFULL * c, 0, N),
                                (1, NFULL * c + 1, 0, N),
                                (2, NFULL * NCORES + c // 2,
                                 (c % 2) * (N // 2), N // 2)):
            ot = yt[s, :, :n_q]                       # [D, n_q], unnormalized
            out[blk, lo:lo + n_q] = (ot / z[s, :n_q]).T
    return out.reshape(B, L, N, D)


def kernel(x, beta, _trace=False, _fast=True, _tmpdir=None):
    x = np.asarray(x, dtype=np.float32)
    assert x.shape == (B, L, D, H, W), x.shape
    beta_f = float(np.asarray(beta))
    prog = _get_program(beta_f, _fast)
    in_maps = make_in_maps(x, _fast)
    res = run_bass_kernel_spmd(prog, in_maps, core_ids=list(range(NCORES)),
                               trace=_trace, tmpdir=_tmpdir)
    out = assemble_output(res.results)
    if _trace:
        return out, res
    return out
